# revision 28
# baseline (speedup 1.0000x reference)
"""Trainium2 Bass kernel for the sparse Lie-bracket bilinear layer.

  out[b, k] = alpha * sum_{t : idx_k[t]==k} coeff[t] * x[b, idx_i[t]] * y[b, idx_j[t]]

Strategy (data-parallel over batch across 8 NeuronCores, no collectives):
  - Host: bucket triples by (i_range(64), j_range(64)) [16 buckets],
    dedupe (i, j) pairs within a bucket (all their k-targets merge into
    the pair's scatter rows), order pairs kh0-only / mixed / kh1-only,
    cut into chunks of 128 pairs.  Most chunks scatter into a single
    k-half; the 1-2 straddler chunks per bucket scatter into both.
  - Per chunk fp16 blocks (pinned in SBUF for the whole kernel):
      G  [128, 128 t]  packed gather one-hots: row strip slot_i*64 for
                       x, strip slot_j*64 for y (slot_i != slot_j via a
                       partition-swapped x copy), so both gather matmuls
                       run concurrently on different PE row strips.
      S0/S1 [128 t, 128 k_local]  coeff-valued scatter matrices.
  - Per chunk: 2 concurrent gather matmuls (PE) -> PSUM, one
    PSUM->SBUF fp16 cast of yj (ACT), vals = xi * yjs fp16 (DVE),
    1-2 scatter matmuls accumulating into persistent PSUM (PE), plus a
    zero-weight dummy matmul (+= 0) that keeps the PE clock gate (HAM)
    from throttling during the DVE/ACT-bound steady state.
"""

import numpy as np

import concourse.bass as bass  # noqa: F401
import concourse.mybir as mybir
from concourse import bacc
from concourse.tile import TileContext
from concourse.bass_utils import run_bass_kernel_spmd

NCORES = 8
P = 128
H = 64
SCAT = 4  # scatter lag (chunks) behind the gather front

_PROG_CACHE = {}

LAST_RESULTS = None  # stash for test.py (exec time / profile)


def _build_program(meta, total_blocks, b_core, bt, n_bt, pos):
    """meta: per chunk (row_ofs_blocks, ir, jr, si, use_flip, has0, has1).
    pos: slab index (0-5) in xyz for blocks xt0 xt1 xf0 xf1 yt0 yt1."""
    nc = bacc.Bacc("TRN2", target_bir_lowering=False, debug=False,
                   num_devices=NCORES)
    f16 = mybir.dt.float16
    f32 = mybir.dt.float32
    n_chunks = len(meta)

    xyz = nc.dram_tensor("xyz", [P, 6 * b_core], f16, kind="ExternalInput")
    w = nc.dram_tensor("w", [P, total_blocks * P], f16, kind="ExternalInput")
    out = nc.dram_tensor("out", [2 * P, b_core], f32, kind="ExternalOutput")

    last_for_half = {}
    for c, m in enumerate(meta):
        if m[5]:
            last_for_half[0] = c
        if m[6]:
            last_for_half[1] = c

    with TileContext(nc) as tc:
        with (
            tc.tile_pool(name="const", bufs=1) as constp,
            tc.tile_pool(name="wpin", bufs=1) as wpin,
            tc.tile_pool(name="vec", bufs=4) as vecp,
            tc.tile_pool(name="gpsum", bufs=3, space="PSUM") as gps,
            tc.tile_pool(name="accp", bufs=1, space="PSUM") as accp,
        ):
            # xyz slab order is permuted so the first chunk's two slabs
            # arrive in the first (small) DMA; the rest follow.
            xyzt = constp.tile([P, 6 * b_core], f16, name="xyz", tag="xyz")
            nc.sync.dma_start(out=xyzt[:, 0:2 * b_core],
                              in_=xyz[:, 0:2 * b_core])
            nc.sync.dma_start(out=xyzt[:, 2 * b_core:],
                              in_=xyz[:, 2 * b_core:])
            zero = constp.tile([P, P], f16, name="zero", tag="zero")
            nc.vector.memset(zero[:], 0.0)

            # Per-chunk weight tiles pinned in SBUF: DMA'd during the b=0
            # pass, reused without DMA in the b=1 pass.
            wts = []
            for c, m in enumerate(meta):
                nblk = 1 + m[5] + m[6]
                wts.append(wpin.tile([P, nblk * P], f16, name=f"w{c}",
                                     tag=f"w{c}"))

            def x_src(ir, si, use_flip, bs):
                base = pos[(2 if use_flip else 0) + ir // 2] * b_core
                return xyzt[si * H:(si + 1) * H,
                            base + bs.start:base + bs.stop]

            def y_src(jr, bs):
                base = pos[4 + jr // 2] * b_core
                sj = jr % 2
                return xyzt[sj * H:(sj + 1) * H,
                            base + bs.start:base + bs.stop]

            for b in range(n_bt):
                bs = slice(b * bt, (b + 1) * bt)
                acc = [accp.tile([P, bt], f32, name="acc0", tag="acc0"),
                       accp.tile([P, bt], f32, name="acc1", tag="acc1")]
                started = [False, False]
                st = {}
                for cc in range(n_chunks + SCAT):
                    if cc < n_chunks:
                        c = cc
                        ofs, ir, jr, si, ufx, has0, has1 = meta[c]
                        sj = jr % 2
                        wt = wts[c]
                        if b == 0:
                            nblk = 1 + has0 + has1
                            nc.sync.dma_start(
                                out=wt[:],
                                in_=w[:, ofs * P:(ofs + nblk) * P])
                        xi = gps.tile([P, bt], f32, tag="xi", bufs=3)
                        yj = gps.tile([P, bt], f32, tag="yj", bufs=3)
                        nc.tensor.matmul(out=xi[:],
                                         lhsT=wt[si * H:(si + 1) * H, 0:P],
                                         rhs=x_src(ir, si, ufx, bs),
                                         start=True, stop=True)
                        nc.tensor.matmul(out=yj[:],
                                         lhsT=wt[sj * H:(sj + 1) * H, 0:P],
                                         rhs=y_src(jr, bs),
                                         start=True, stop=True)
                        yjs = vecp.tile([P, bt], f16, tag="yjs", bufs=3)
                        nc.scalar.copy(out=yjs[:], in_=yj[:])
                        st[c] = {"xi": xi, "yjs": yjs}

                    c1 = cc - 1
                    if 0 <= c1 < n_chunks:
                        s = st[c1]
                        vals = vecp.tile([P, bt], f16, tag="vals", bufs=6)
                        nc.vector.tensor_tensor(
                            out=vals[:], in0=s["xi"][:], in1=s["yjs"][:],
                            op=mybir.AluOpType.mult)
                        s["vals"] = vals

                    cs = cc - SCAT
                    if cs >= 0:
                        s = st.pop(cs)
                        _, _, _, _, _, has0, has1 = meta[cs]
                        wt = wts[cs]
                        khd = None
                        for kh, has in ((0, has0), (1, has1)):
                            if not has:
                                continue
                            blk = 1 if kh == 0 or not has0 else 2
                            nc.tensor.matmul(out=acc[kh][:],
                                             lhsT=wt[:, blk * P:(blk + 1) * P],
                                             rhs=s["vals"][:],
                                             start=not started[kh],
                                             stop=(cs == last_for_half[kh]))
                            started[kh] = True
                            if cs < last_for_half[kh]:
                                khd = kh
                            else:
                                # kh closed: drain it now so the output
                                # copy + DMA overlap the remaining chunks.
                                osb = vecp.tile([P, bt], f32, tag="osb",
                                                bufs=2)
                                nc.scalar.copy(out=osb[:], in_=acc[kh][:])
                                nc.sync.dma_start(
                                    out=out[kh * P:(kh + 1) * P, bs],
                                    in_=osb[:])
                        # HAM warm-keeper: zero-weight accumulate (+= 0)
                        # fills the PE idle slot so the clock gate never
                        # throttles during the DVE/ACT-bound steady state.
                        # Straddler chunks already fill it with their
                        # second scatter.
                        if khd is not None and not (has0 and has1):
                            nc.tensor.matmul(out=acc[khd][:, 0:P],
                                             lhsT=zero[:],
                                             rhs=xyzt[:, b * bt:b * bt + P],
                                             start=False, stop=False)

    nc.compile()
    return nc


def _flip_ranges(a):
    """Swap the two 64-row ranges inside each 128-row half."""
    return np.concatenate([a[H:2 * H], a[0:H], a[3 * H:4 * H], a[2 * H:3 * H]])


def _prep_chunks(ii, jj, kk, cc):
    """Bucket by (i_range, j_range), dedupe (i, j) pairs, order pairs
    kh0/mixed/kh1, cut into chunks of 128.  Returns (w, meta) with
    w = concatenated [128, 128] fp16 blocks (G | S0? | S1?) per chunk and
    meta entries (row_ofs_blocks, ir, jr, si, use_flip, has0, has1)."""
    bucket = (ii // H) * 4 + (jj // H)
    chunks = []
    for bkt in range(16):
        sel = np.nonzero(bucket == bkt)[0]
        if len(sel) == 0:
            continue
        ir, jr = bkt // 4, bkt % 4
        sj = jr % 2
        use_flip = (ir % 2 == sj)
        si = 1 - sj if use_flip else ir % 2
        groups = {}
        for t in sel:
            groups.setdefault((ii[t], jj[t]), []).append(t)
        # order: kh0-only pairs, mixed, kh1-only
        k0, kmix, k1 = [], [], []
        for key, ts in groups.items():
            khs = {kk[t] // P for t in ts}
            (k0 if khs == {0} else k1 if khs == {1} else kmix).append(key)
        keys = k0 + kmix + k1
        for gs in range(0, len(keys), P):
            gk = keys[gs:gs + P]
            G = np.zeros((P, P), np.float64)
            S = [np.zeros((P, P), np.float64), np.zeros((P, P), np.float64)]
            has = [False, False]
            for tl, (iv, jv) in enumerate(gk):
                G[si * H + (iv - ir * H), tl] = 1.0
                G[sj * H + (jv - jr * H), tl] = 1.0
                for torig in groups[(iv, jv)]:
                    kh = kk[torig] // P
                    S[kh][tl, kk[torig] - kh * P] += cc[torig]
                    has[kh] = True
            for tl in range(len(gk), P):  # pad columns: harmless one-hots
                G[si * H, tl] = 1.0
                G[sj * H, tl] = 1.0
            chunk_blocks = [G] + [S[kh] for kh in range(2) if has[kh]]
            chunks.append((chunk_blocks,
                           (ir, jr, si, use_flip, has[0], has[1])))
    # order: kh0-only chunks, straddlers, kh1-only -- so acc0 closes well
    # before the end and its output drain overlaps the kh1 tail.
    chunks.sort(key=lambda ch: 1 if (ch[1][4] and ch[1][5])
                else 0 if ch[1][4] else 2)
    meta = []
    blocks = []
    ofs = 0
    for chunk_blocks, m in chunks:
        blocks += chunk_blocks
        meta.append((ofs,) + m)
        ofs += len(chunk_blocks)
    w = np.concatenate(blocks, axis=1).astype(np.float16)
    return w, meta, ofs


def kernel(x, y, idx_i, idx_j, idx_k, coeff, alpha):
    global LAST_RESULTS
    x = np.asarray(x, dtype=np.float32)
    y = np.asarray(y, dtype=np.float32)
    ii = np.asarray(idx_i).astype(np.int64)
    jj = np.asarray(idx_j).astype(np.int64)
    kk = np.asarray(idx_k).astype(np.int64)
    cc = (np.asarray(coeff).astype(np.float64)
          * np.float64(np.asarray(alpha).reshape(-1)[0]))

    B, ALG = x.shape
    assert ALG <= 2 * P
    assert B % NCORES == 0
    b_core = B // NCORES
    bt = min(512, b_core)
    assert b_core % bt == 0
    n_bt = b_core // bt

    w, meta, total_blocks = _prep_chunks(ii, jj, kk, cc)

    # xyz slab permutation: the first chunk's x- and y-slabs go first so
    # the split DMA delivers them early.  Slab ids: xt0 xt1 xf0 xf1 yt0 yt1.
    _, ir0, jr0, _, ufx0, _, _ = meta[0]
    first = [(2 if ufx0 else 0) + ir0 // 2, 4 + jr0 // 2]
    perm = first + [s for s in range(6) if s not in first]
    pos = tuple(perm.index(s) for s in range(6))

    key = (tuple(meta), total_blocks, b_core, bt, n_bt, pos)
    if key not in _PROG_CACHE:
        _PROG_CACHE[key] = _build_program(meta, total_blocks, b_core, bt,
                                          n_bt, pos)
    nc = _PROG_CACHE[key]

    # ---- per-core inputs ----
    in_maps = []
    pad_rows = 2 * P - ALG
    for m in range(NCORES):
        xs = x[m * b_core:(m + 1) * b_core].T
        ys = y[m * b_core:(m + 1) * b_core].T
        xs = np.concatenate([xs, np.zeros((pad_rows, b_core), np.float32)], 0)
        ys = np.concatenate([ys, np.zeros((pad_rows, b_core), np.float32)], 0)
        xt = xs.astype(np.float16)
        xf = _flip_ranges(xt)
        yt = ys.astype(np.float16)
        slabs = [xt[0:P], xt[P:2 * P], xf[0:P], xf[P:2 * P],
                 yt[0:P], yt[P:2 * P]]
        xyz = np.concatenate([slabs[s] for s in perm], axis=1)
        in_maps.append({"xyz": xyz, "w": w})

    res = run_bass_kernel_spmd(nc, in_maps, core_ids=list(range(NCORES)))
    LAST_RESULTS = res

    outp = np.empty((B, ALG), np.float32)
    for m in range(NCORES):
        outp[m * b_core:(m + 1) * b_core] = res.results[m]["out"][:ALG].T
    return outp


# revision 29
# speedup vs baseline: 1.0276x; 1.0276x over previous
"""Trainium2 Bass kernel for the sparse Lie-bracket bilinear layer.

  out[b, k] = alpha * sum_{t : idx_k[t]==k} coeff[t] * x[b, idx_i[t]] * y[b, idx_j[t]]

Strategy (data-parallel over batch across 8 NeuronCores, no collectives):
  - Host: bucket triples by (i_range(64), j_range(64)) [16 buckets],
    dedupe (i, j) pairs within a bucket (all their k-targets merge into
    the pair's scatter rows), order pairs kh0-only / mixed / kh1-only,
    cut into chunks of 128 pairs.  Most chunks scatter into a single
    k-half; the 1-2 straddler chunks per bucket scatter into both.
  - Per chunk fp16 blocks (pinned in SBUF for the whole kernel):
      G  [128, 128 t]  packed gather one-hots: row strip slot_i*64 for
                       x, strip slot_j*64 for y (slot_i != slot_j via a
                       partition-swapped x copy), so both gather matmuls
                       run concurrently on different PE row strips.
      S0/S1 [128 t, 128 k_local]  coeff-valued scatter matrices.
  - Per chunk: 2 concurrent gather matmuls (PE) -> PSUM, one
    PSUM->SBUF fp16 cast of yj (ACT), vals = xi * yjs fp16 (DVE),
    1-2 scatter matmuls accumulating into persistent PSUM (PE), plus a
    zero-weight dummy matmul (+= 0) that keeps the PE clock gate (HAM)
    from throttling during the DVE/ACT-bound steady state.
"""

import numpy as np

import concourse.bass as bass  # noqa: F401
import concourse.mybir as mybir
from concourse import bacc
from concourse.tile import TileContext
from concourse.bass_utils import run_bass_kernel_spmd

NCORES = 8
P = 128
H = 64
SCAT = 4  # scatter lag (chunks) behind the gather front

_PROG_CACHE = {}

LAST_RESULTS = None  # stash for test.py (exec time / profile)


def _build_program(meta, total_blocks, b_core, bt, n_bt, pos):
    """meta: per chunk (row_ofs_blocks, ir, jr, si, use_flip, has0, has1).
    pos: slab index (0-5) in xyz for blocks xt0 xt1 xf0 xf1 yt0 yt1."""
    nc = bacc.Bacc("TRN2", target_bir_lowering=False, debug=False,
                   num_devices=NCORES)
    f16 = mybir.dt.float16
    f32 = mybir.dt.float32
    n_chunks = len(meta)

    xyz = nc.dram_tensor("xyz", [P, 6 * b_core], f16, kind="ExternalInput")
    w = nc.dram_tensor("w", [P, total_blocks * P], f16, kind="ExternalInput")
    out = nc.dram_tensor("out", [2 * P, b_core], f32, kind="ExternalOutput")

    last_for_half = {}
    for c, m in enumerate(meta):
        if m[5]:
            last_for_half[0] = c
        if m[6]:
            last_for_half[1] = c

    with TileContext(nc) as tc:
        with (
            tc.tile_pool(name="const", bufs=1) as constp,
            tc.tile_pool(name="wpin", bufs=1) as wpin,
            tc.tile_pool(name="vec", bufs=4) as vecp,
            tc.tile_pool(name="gpsum", bufs=3, space="PSUM") as gps,
            tc.tile_pool(name="accp", bufs=1, space="PSUM") as accp,
        ):
            # xyz slab order is permuted so the first chunk's two slabs
            # arrive in the first (small) DMA; the rest follow.
            xyzt = constp.tile([P, 6 * b_core], f16, name="xyz", tag="xyz")
            nc.sync.dma_start(out=xyzt[:, 0:2 * b_core],
                              in_=xyz[:, 0:2 * b_core])
            nc.sync.dma_start(out=xyzt[:, 2 * b_core:],
                              in_=xyz[:, 2 * b_core:])
            zero = constp.tile([P, P], f16, name="zero", tag="zero")
            nc.vector.memset(zero[:], 0.0)

            # Per-chunk weight tiles pinned in SBUF: DMA'd during the b=0
            # pass, reused without DMA in the b=1 pass.
            wts = []
            for c, m in enumerate(meta):
                nblk = 1 + m[5] + m[6]
                wts.append(wpin.tile([P, nblk * P], f16, name=f"w{c}",
                                     tag=f"w{c}"))

            def x_src(ir, si, use_flip, bs):
                base = pos[(2 if use_flip else 0) + ir // 2] * b_core
                return xyzt[si * H:(si + 1) * H,
                            base + bs.start:base + bs.stop]

            def y_src(jr, bs):
                base = pos[4 + jr // 2] * b_core
                sj = jr % 2
                return xyzt[sj * H:(sj + 1) * H,
                            base + bs.start:base + bs.stop]

            for b in range(n_bt):
                bs = slice(b * bt, (b + 1) * bt)
                acc = [accp.tile([P, bt], f32, name="acc0", tag="acc0"),
                       accp.tile([P, bt], f32, name="acc1", tag="acc1")]
                started = [False, False]
                st = {}
                for cc in range(n_chunks + SCAT):
                    if cc < n_chunks:
                        c = cc
                        ofs, ir, jr, si, ufx, has0, has1 = meta[c]
                        sj = jr % 2
                        wt = wts[c]
                        if b == 0:
                            nblk = 1 + has0 + has1
                            nc.sync.dma_start(
                                out=wt[:],
                                in_=w[:, ofs * P:(ofs + nblk) * P])
                        xi = gps.tile([P, bt], f32, tag="xi", bufs=3)
                        yj = gps.tile([P, bt], f32, tag="yj", bufs=3)
                        nc.tensor.matmul(out=xi[:],
                                         lhsT=wt[si * H:(si + 1) * H, 0:P],
                                         rhs=x_src(ir, si, ufx, bs),
                                         start=True, stop=True)
                        nc.tensor.matmul(out=yj[:],
                                         lhsT=wt[sj * H:(sj + 1) * H, 0:P],
                                         rhs=y_src(jr, bs),
                                         start=True, stop=True)
                        yjs = vecp.tile([P, bt], f16, tag="yjs", bufs=3)
                        nc.scalar.copy(out=yjs[:], in_=yj[:])
                        st[c] = {"xi": xi, "yjs": yjs}

                    c1 = cc - 1
                    if 0 <= c1 < n_chunks:
                        s = st[c1]
                        vals = vecp.tile([P, bt], f16, tag="vals", bufs=6)
                        nc.vector.tensor_tensor(
                            out=vals[:], in0=s["xi"][:], in1=s["yjs"][:],
                            op=mybir.AluOpType.mult)
                        s["vals"] = vals

                    cs = cc - SCAT
                    if cs >= 0:
                        s = st.pop(cs)
                        _, _, _, _, _, has0, has1 = meta[cs]
                        wt = wts[cs]
                        khd = None
                        for kh, has in ((0, has0), (1, has1)):
                            if not has:
                                continue
                            blk = 1 if kh == 0 or not has0 else 2
                            nc.tensor.matmul(out=acc[kh][:],
                                             lhsT=wt[:, blk * P:(blk + 1) * P],
                                             rhs=s["vals"][:],
                                             start=not started[kh],
                                             stop=(cs == last_for_half[kh]))
                            started[kh] = True
                            if cs < last_for_half[kh]:
                                khd = kh
                            else:
                                # kh closed: drain it now so the output
                                # copy + DMA overlap the remaining chunks.
                                osb = vecp.tile([P, bt], f32, tag="osb",
                                                bufs=2)
                                nc.scalar.copy(out=osb[:], in_=acc[kh][:])
                                nc.sync.dma_start(
                                    out=out[kh * P:(kh + 1) * P, bs],
                                    in_=osb[:])
                        # HAM warm-keeper: zero-weight accumulate (+= 0)
                        # fills the PE idle slot so the clock gate never
                        # throttles during the DVE/ACT-bound steady state.
                        # Straddler chunks already fill it with their
                        # second scatter.
                        if khd is not None and not (has0 and has1):
                            nc.tensor.matmul(out=acc[khd][:, 0:P],
                                             lhsT=zero[:],
                                             rhs=xyzt[:, b * bt:b * bt + P],
                                             start=False, stop=False)

    nc.compile()
    return nc


def _flip_ranges(a):
    """Swap the two 64-row ranges inside each 128-row half."""
    return np.concatenate([a[H:2 * H], a[0:H], a[3 * H:4 * H], a[2 * H:3 * H]])


def _prep_chunks(ii, jj, kk, cc):
    """Bucket by (i_range, j_range), dedupe (i, j) pairs, order pairs
    kh0/mixed/kh1, cut into chunks of 128.  Returns (w, meta) with
    w = concatenated [128, 128] fp16 blocks (G | S0? | S1?) per chunk and
    meta entries (row_ofs_blocks, ir, jr, si, use_flip, has0, has1)."""
    bucket = (ii // H) * 4 + (jj // H)
    chunks = []
    for bkt in range(16):
        sel = np.nonzero(bucket == bkt)[0]
        if len(sel) == 0:
            continue
        ir, jr = bkt // 4, bkt % 4
        sj = jr % 2
        use_flip = (ir % 2 == sj)
        si = 1 - sj if use_flip else ir % 2
        groups = {}
        for t in sel:
            groups.setdefault((ii[t], jj[t]), []).append(t)
        # order: kh0-only pairs, mixed, kh1-only
        k0, kmix, k1 = [], [], []
        for key, ts in groups.items():
            khs = {kk[t] // P for t in ts}
            (k0 if khs == {0} else k1 if khs == {1} else kmix).append(key)
        keys = k0 + kmix + k1
        for gs in range(0, len(keys), P):
            gk = keys[gs:gs + P]
            G = np.zeros((P, P), np.float64)
            S = [np.zeros((P, P), np.float64), np.zeros((P, P), np.float64)]
            has = [False, False]
            for tl, (iv, jv) in enumerate(gk):
                G[si * H + (iv - ir * H), tl] = 1.0
                G[sj * H + (jv - jr * H), tl] = 1.0
                for torig in groups[(iv, jv)]:
                    kh = kk[torig] // P
                    S[kh][tl, kk[torig] - kh * P] += cc[torig]
                    has[kh] = True
            for tl in range(len(gk), P):  # pad columns: harmless one-hots
                G[si * H, tl] = 1.0
                G[sj * H, tl] = 1.0
            chunk_blocks = [G] + [S[kh] for kh in range(2) if has[kh]]
            chunks.append((chunk_blocks,
                           (ir, jr, si, use_flip, has[0], has[1])))
    meta = []
    blocks = []
    ofs = 0
    for chunk_blocks, m in chunks:
        blocks += chunk_blocks
        meta.append((ofs,) + m)
        ofs += len(chunk_blocks)
    w = np.concatenate(blocks, axis=1).astype(np.float16)
    return w, meta, ofs


def kernel(x, y, idx_i, idx_j, idx_k, coeff, alpha):
    global LAST_RESULTS
    x = np.asarray(x, dtype=np.float32)
    y = np.asarray(y, dtype=np.float32)
    ii = np.asarray(idx_i).astype(np.int64)
    jj = np.asarray(idx_j).astype(np.int64)
    kk = np.asarray(idx_k).astype(np.int64)
    cc = (np.asarray(coeff).astype(np.float64)
          * np.float64(np.asarray(alpha).reshape(-1)[0]))

    B, ALG = x.shape
    assert ALG <= 2 * P
    assert B % NCORES == 0
    b_core = B // NCORES
    bt = min(512, b_core)
    assert b_core % bt == 0
    n_bt = b_core // bt

    w, meta, total_blocks = _prep_chunks(ii, jj, kk, cc)

    # xyz slab permutation: the first chunk's x- and y-slabs go first so
    # the split DMA delivers them early.  Slab ids: xt0 xt1 xf0 xf1 yt0 yt1.
    _, ir0, jr0, _, ufx0, _, _ = meta[0]
    first = [(2 if ufx0 else 0) + ir0 // 2, 4 + jr0 // 2]
    perm = first + [s for s in range(6) if s not in first]
    pos = tuple(perm.index(s) for s in range(6))

    key = (tuple(meta), total_blocks, b_core, bt, n_bt, pos)
    if key not in _PROG_CACHE:
        _PROG_CACHE[key] = _build_program(meta, total_blocks, b_core, bt,
                                          n_bt, pos)
    nc = _PROG_CACHE[key]

    # ---- per-core inputs ----
    in_maps = []
    pad_rows = 2 * P - ALG
    for m in range(NCORES):
        xs = x[m * b_core:(m + 1) * b_core].T
        ys = y[m * b_core:(m + 1) * b_core].T
        xs = np.concatenate([xs, np.zeros((pad_rows, b_core), np.float32)], 0)
        ys = np.concatenate([ys, np.zeros((pad_rows, b_core), np.float32)], 0)
        xt = xs.astype(np.float16)
        xf = _flip_ranges(xt)
        yt = ys.astype(np.float16)
        slabs = [xt[0:P], xt[P:2 * P], xf[0:P], xf[P:2 * P],
                 yt[0:P], yt[P:2 * P]]
        xyz = np.concatenate([slabs[s] for s in perm], axis=1)
        in_maps.append({"xyz": xyz, "w": w})

    res = run_bass_kernel_spmd(nc, in_maps, core_ids=list(range(NCORES)))
    LAST_RESULTS = res

    outp = np.empty((B, ALG), np.float32)
    for m in range(NCORES):
        outp[m * b_core:(m + 1) * b_core] = res.results[m]["out"][:ALG].T
    return outp
